# revision 41
# baseline (speedup 1.0000x reference)
# Mistral sliding-window attention (B=1, S=2048, H=4096, 32 q heads / 8 kv
# heads, window 4096 -> plain causal at this S) on 8 Trainium2 NeuronCores.
#
# Sharding: tensor-parallel over heads with NO on-device collectives. Core c
# owns q heads 4c..4c+3 and kv head c. hidden_states is replicated
# (transposed on host to [H, S]). Each core computes attention for its 4
# heads and then a PARTIAL o_proj over the FULL 4096 output columns using
# only its own 512 attention dims; the host sums the 8 partial outputs.
# This removes the AllGather serialization entirely.
#
# All matmul operands are bf16 (psum accumulation stays fp32): same PE
# stream rate as fp32r but half the DMA/SBUF traffic and half the
# LDWEIGHTS time. Scores are computed transposed (S.T[kv, q]); softmax
# denominators come from an all-ones stationary matmul accumulated
# alongside P@V; the causal mask is a host-precomputed staircase slice
# multiplied in after exp. The attention units (head, chunk) are
# software-pipelined: scores+exp of unit u+1 are emitted before P@V of
# unit u, and the o_proj of chunk c runs right after its 4 heads finish,
# draining psum->sbuf (bf16) -> DRAM.

from contextlib import ExitStack

import numpy as np
import ml_dtypes

import concourse.bacc as bacc
import concourse.bass as bass
import concourse.mybir as mybir
import concourse.tile as tile
from concourse.bass_utils import run_bass_kernel_spmd
from concourse.masks import make_identity

HIDDEN = 4096
NH = 32
NKV = 8
HD = 128
THETA = 10000.0
S = 2048
NCORES = 8

QH = NH // NCORES          # 4 q heads per core
DQ = QH * HD               # 512 (per-core attn width)
DOUT = DQ + 2 * HD         # 768 = q heads + k + v projection width
MT = DOUT // 128           # 6 projection m-tiles (0..3 q, 4 k, 5 v)
KT = HIDDEN // 128         # 32 contraction tiles
KG = 4                     # x-load group: k-tiles per DMA
TCH = 512                  # token chunk (matmul moving dim)
NTCH = S // TCH            # 4
KVT = S // 128             # 16 kv tiles
HG = HIDDEN // TCH         # 8 o_proj output column groups
SCALE = 1.0 / float(np.sqrt(HD))

F32 = mybir.dt.float32
BF16 = mybir.dt.bfloat16
FP8E4 = mybir.dt.float8e4
FP8E5 = mybir.dt.float8e5
EXP = mybir.ActivationFunctionType.Exp
CSHIFT = 1.0   # exp(s - C): keeps e5m2 probabilities in range (max logit ~11.3)


def _rope(nc, pool, src, dst, cs, sn):
    """dst = src*cos + rotate_half(src)*sin, in [d, tok] layout.

    src/dst are [128, n]. cs is cos duplicated into both 64-row halves;
    sn is [+sin; -sin]. Then with B = src*sn and Bx = halves-swapped B:
    dst = src*cs + Bx reproduces rope with only whole-tile (partition-
    aligned) DVE ops plus two small SBUF->SBUF swap DMAs.
    """
    A = pool.tile([128, TCH], F32, name="ropeA")
    B = pool.tile([128, TCH], F32, name="ropeB")
    Bx = pool.tile([128, TCH], F32, name="ropeBx")
    nc.vector.tensor_mul(A, src, cs)
    nc.vector.tensor_mul(B, src, sn)
    nc.sync.dma_start(out=Bx[0:64, :], in_=B[64:128, :])
    nc.sync.dma_start(out=Bx[64:128, :], in_=B[0:64, :])
    nc.vector.tensor_add(dst, A, Bx)


def build_kernel_body(ctx: ExitStack, tc: tile.TileContext, outs, ins):
    nc = tc.nc
    xT, wqkv, ow, cos_t, sin_t, stair = (
        ins["xT"], ins["wqkv"], ins["ow"], ins["cos_t"], ins["sin_t"], ins["stair"],
    )
    out = outs["out"]

    # ONE pool scope for the whole kernel: closing a pool between the proj
    # and attention phases emits an all-engine barrier that stalls the PE
    # ~16us on the last chunk's serial rope chain. To fit PSUM (8 banks),
    # the projection processes its 6 m-tiles as two groups of 3 banks
    # (psA/psB/psC time-multiplexed), psums are staged to SBUF by a fast
    # scalar copy so rope never holds a bank, and the attention/oproj tiles
    # reuse the proj banks via explicit tags:
    #   psA psB psC     : proj chains -> attention po (A/B) + den (C)
    #   sc0 sc1 sc2 sc3 : score tiles -> oproj chains
    #   den1            : attention den (odd units)
    singles = ctx.enter_context(tc.tile_pool(name="singles", bufs=1))
    wp = ctx.enter_context(tc.tile_pool(name="wq", bufs=1))
    xp = ctx.enter_context(tc.tile_pool(name="xt", bufs=1))
    rps = ctx.enter_context(tc.tile_pool(name="ropes", bufs=1))
    rpv = ctx.enter_context(tc.tile_pool(name="ropev", bufs=2))
    ptp = ctx.enter_context(tc.tile_pool(name="pt", bufs=16))
    aop = ctx.enter_context(tc.tile_pool(name="ao", bufs=8))
    rcp = ctx.enter_context(tc.tile_pool(name="rc", bufs=2))
    obp = ctx.enter_context(tc.tile_pool(name="ob", bufs=4))
    pq = ctx.enter_context(tc.tile_pool(name="pq", bufs=1, space="PSUM"))

    stair_sb = singles.tile([128, 896], BF16)
    ones_sb = singles.tile([128, 128], BF16)
    ow_sb = singles.tile([128, QH, HIDDEN], BF16)   # [d, head, hid]
    qTc = [singles.tile([128, QH, TCH], BF16, name=f"qT{t}")
           for t in range(NTCH)]
    kTc = [singles.tile([128, TCH], BF16, name=f"kT{t}")
           for t in range(NTCH)]
    Vc = [singles.tile([128, 4, HD], BF16, name=f"V{t}")
          for t in range(NTCH)]                     # Vc[t][:, j%4, :] = [tok, d]

    cos_sb = wp.tile([128, S], BF16)
    sin_sb = wp.tile([128, S], BF16)
    vT = wp.tile([128, S], BF16)
    nc.vector.memset(ones_sb, 1.0)

    # ---- phase 1: QKV projection + RoPE --------------------------------
    wq3 = wqkv.rearrange("(a p) d -> p a d", p=128)   # [128, KT, DOUT]
    x3 = xT.rearrange("(k p) s -> p k s", p=128)
    NG = KT // KG                                     # 8 x-load groups
    w_sb = [wp.tile([128, DOUT], BF16, name=f"w{k}", tag=f"w{k}")
            for k in range(KT)]
    # interleave t=0 x-group loads with the weight k-tiles so neither
    # starves the first matmul chain; cos/sin next; stair/ow later
    xg_t0 = [xp.tile([128, KG, TCH], BF16, name="xg", tag=f"xg{g}")
             for g in range(NG)]
    nc.sync.dma_start(out=w_sb[0], in_=wq3[:, 0, :])
    nc.sync.dma_start(out=xg_t0[0], in_=x3[:, 0:KG, 0:TCH])
    for kg in range(NG):
        for k in range(kg * KG, (kg + 1) * KG):
            if k == 0:
                continue
            nc.sync.dma_start(out=w_sb[k], in_=wq3[:, k, :])
        if kg + 1 < NG:
            nc.sync.dma_start(
                out=xg_t0[kg + 1],
                in_=x3[:, (kg + 1) * KG:(kg + 2) * KG, 0:TCH])
    nc.sync.dma_start(out=cos_sb, in_=cos_t)
    nc.sync.dma_start(out=sin_sb, in_=sin_t)

    for t in range(NTCH):
        if t == 0:
            xgs = xg_t0
        else:
            xgs = []
            for kg in range(NG):
                xg = xp.tile([128, KG, TCH], BF16, name="xg", tag=f"xg{kg}")
                nc.sync.dma_start(
                    out=xg,
                    in_=x3[:, kg * KG:(kg + 1) * KG, t * TCH:(t + 1) * TCH])
                xgs.append(xg)
        cs = cos_sb[:, t * TCH:(t + 1) * TCH]
        sn = sin_sb[:, t * TCH:(t + 1) * TCH]
        for gi, grp in enumerate(((0, 1, 2), (3, 4, 5))):
            pst = [pq.tile([128, TCH], F32, name=f"ps{mi}", tag=f"ps{'ABC'[mi]}")
                   for mi in range(3)]
            for kg in range(NG):
                for ki in range(KG):
                    k = kg * KG + ki
                    for mi, m in enumerate(grp):
                        nc.tensor.matmul(
                            pst[mi],
                            lhsT=w_sb[k][:, m * 128:(m + 1) * 128],
                            rhs=xgs[kg][:, ki, :],
                            start=(k == 0), stop=(k == KT - 1),
                        )
            # stage psum -> sbuf with a single fast scalar copy so rope's
            # four slow DVE reads never hold the bank
            if gi == 0:
                for mi in range(3):
                    stg = rps.tile([128, TCH], F32, name="stg", tag=f"sg{mi}")
                    nc.scalar.copy(out=stg, in_=pst[mi])
                    _rope(nc, rpv, stg, qTc[t][:, mi, :], cs, sn)
            else:
                stg = rps.tile([128, TCH], F32, name="stg", tag="sg0")
                nc.scalar.copy(out=stg, in_=pst[0])
                _rope(nc, rpv, stg, qTc[t][:, 3, :], cs, sn)
                stg2 = rps.tile([128, TCH], F32, name="stg", tag="sg1")
                nc.scalar.copy(out=stg2, in_=pst[1])
                _rope(nc, rpv, stg2, kTc[t], cs, sn)
                nc.scalar.copy(out=vT[:, t * TCH:(t + 1) * TCH], in_=pst[2])
                for j in range(4):
                    jj = 4 * t + j
                    nc.sync.dma_start_transpose(
                        out=Vc[t][:, j, :],
                        in_=vT[:, jj * 128:(jj + 1) * 128])
        if t == 0:
            nc.sync.dma_start(out=stair_sb, in_=stair)
            nc.sync.dma_start(out=ow_sb, in_=ow)

    # ---- phase 2: attention + partial o_proj, software-pipelined -------
    def attn_S(h, c):
        """Scores + exp + causal stair; diagonal tiles only compute the
        unmasked q-column range."""
        pts = []
        qslice = qTc[c][:, h, :]
        for j in range(4 * c + 4):
            rdiag = j - 4 * c
            q0 = rdiag * 128 if rdiag > 0 else 0
            sc = pq.tile([128, TCH], F32, name="sc", tag=f"sc{j % 4}")
            nc.tensor.matmul(sc[:, q0:],
                             lhsT=kTc[j // 4][:, (j % 4) * 128:
                                              (j % 4 + 1) * 128],
                             rhs=qslice[:, q0:], start=True, stop=True)
            pt = ptp.tile([128, TCH], BF16, name="pt", tag="pt")
            nc.scalar.activation(pt[:, q0:], sc[:, q0:], EXP, scale=SCALE)
            if rdiag >= 0:  # tile touches the causal diagonal
                nc.vector.tensor_mul(pt[:, q0:], pt[:, q0:],
                                     stair_sb[:, 384:384 + TCH - q0])
            pts.append((pt, q0))
        return pts

    def attn_PV(u, h, c, pts):
        """P@V + denominator + normalize for one (head, q-chunk)."""
        jmax = 4 * c + 3
        po = pq.tile([128, TCH], F32, name="po", tag=f"ps{'AB'[u % 2]}")
        den = pq.tile([128, TCH], F32, name="den",
                      tag=("psC" if u % 2 == 0 else "den1"))
        for j, (pt, q0) in enumerate(pts):
            nc.tensor.matmul(po[:, q0:], lhsT=Vc[j // 4][:, j % 4, :],
                             rhs=pt[:, q0:],
                             start=(j == 0), stop=(j == jmax))
            nc.tensor.matmul(den[:, q0:], lhsT=ones_sb, rhs=pt[:, q0:],
                             start=(j == 0), stop=(j == jmax))
        rec = rcp.tile([128, TCH], F32, name="rec")
        nc.vector.reciprocal_approx_fast(rec, den)
        ao = aop.tile([128, TCH], BF16, name="ao")
        nc.vector.tensor_mul(ao, po, rec)
        return ao

    def oproj(c, aos):
        """Partial o_proj for token chunk c: out[tok, :] over all 4096
        columns, contracting this core's 4 heads (512 attn dims)."""
        idx = 0
        for ts in range(TCH // 128):
            for hg in range(HG):
                op = pq.tile([128, TCH], F32, name="op", tag=f"sc{idx % 4}")
                idx += 1
                for h in range(QH):
                    nc.tensor.matmul(
                        op,
                        lhsT=aos[h][:, ts * 128:(ts + 1) * 128],
                        rhs=ow_sb[:, h, hg * TCH:(hg + 1) * TCH],
                        start=(h == 0), stop=(h == QH - 1),
                    )
                ob = obp.tile([128, TCH], BF16, name="ob")
                nc.vector.tensor_copy(ob, op)
                r0 = c * TCH + ts * 128
                nc.sync.dma_start(
                    out=out[r0:r0 + 128, hg * TCH:(hg + 1) * TCH], in_=ob)

    units = [(c, h) for c in range(NTCH) for h in range(QH)]
    pts_cur = attn_S(units[0][1], units[0][0])
    aos = []
    for idx, (c, h) in enumerate(units):
        if idx + 1 < len(units):
            c2, h2 = units[idx + 1]
            pts_nxt = attn_S(h2, c2)
        else:
            pts_nxt = None
        aos.append(attn_PV(idx, h, c, pts_cur))
        pts_cur = pts_nxt
        if h == QH - 1:
            oproj(c, aos)
            aos = []


_NC_CACHE = None


def build_program():
    global _NC_CACHE
    if _NC_CACHE is not None:
        return _NC_CACHE
    nc = bacc.Bacc("TRN2", target_bir_lowering=False, debug=False,
                   num_devices=NCORES)
    ins = {
        "xT": nc.dram_tensor("xT", [HIDDEN, S], BF16, kind="ExternalInput").ap(),
        "wqkv": nc.dram_tensor("wqkv", [HIDDEN, DOUT], BF16,
                               kind="ExternalInput").ap(),
        "ow": nc.dram_tensor("ow", [128, QH, HIDDEN], BF16,
                             kind="ExternalInput").ap(),
        "cos_t": nc.dram_tensor("cos_t", [128, S], BF16,
                                kind="ExternalInput").ap(),
        "sin_t": nc.dram_tensor("sin_t", [128, S], BF16,
                                kind="ExternalInput").ap(),
        "stair": nc.dram_tensor("stair", [128, 896], BF16,
                                kind="ExternalInput").ap(),
    }
    outs = {"out": nc.dram_tensor("out", [S, HIDDEN], BF16,
                                  kind="ExternalOutput").ap()}
    with tile.TileContext(nc) as tc:
        with ExitStack() as ctx:
            build_kernel_body(ctx, tc, outs, ins)
    nc.compile()
    _NC_CACHE = nc
    return nc


def make_in_maps(hidden_states, position_ids, q_w, k_w, v_w, o_w):
    bf16 = ml_dtypes.bfloat16
    x = np.asarray(hidden_states, dtype=np.float32).reshape(S, HIDDEN)
    xT = np.ascontiguousarray(x.T).astype(bf16)
    pos = np.asarray(position_ids).reshape(S).astype(np.float64)
    inv = 1.0 / (THETA ** (np.arange(0, HD, 2, dtype=np.float64) / HD))
    fr = inv[:, None] * pos[None, :]                       # [64, S]
    # cos duplicated into both 64-row halves; sin stored [+sin; -sin] so
    # rope becomes whole-tile muls + a halves-swap (see _rope)
    c64 = np.cos(fr)
    s64 = np.sin(fr)
    cos_t = np.concatenate([c64, c64], axis=0).astype(bf16)    # [128, S]
    sin_t = np.concatenate([s64, -s64], axis=0).astype(bf16)   # [128, S]
    u = np.arange(896, dtype=np.int64)[None, :]
    kvi = np.arange(128, dtype=np.int64)[:, None]
    stair = ((u - kvi) >= 384).astype(bf16)                # [128, 896]

    q_w = np.asarray(q_w, dtype=np.float32)
    k_w = np.asarray(k_w, dtype=np.float32)
    v_w = np.asarray(v_w, dtype=np.float32)
    o_w = np.asarray(o_w, dtype=np.float32)

    in_maps = []
    for c in range(NCORES):
        wqkv = np.ascontiguousarray(np.concatenate(
            [q_w[:, c * DQ:(c + 1) * DQ],
             k_w[:, c * HD:(c + 1) * HD],
             v_w[:, c * HD:(c + 1) * HD]], axis=1)).astype(bf16)
        # o_w rows for this core's 512 attn dims -> [d 128, head 4, hid 4096]
        owc = np.ascontiguousarray(
            o_w[c * DQ:(c + 1) * DQ, :].reshape(QH, HD, HIDDEN)
            .transpose(1, 0, 2)).astype(bf16)
        in_maps.append({"xT": xT, "wqkv": wqkv, "ow": owc,
                        "cos_t": cos_t, "sin_t": sin_t, "stair": stair})
    return in_maps


def run(inputs: dict, trace: bool = False):
    """Run on the 8 NeuronCores; returns (full_output, BassKernelResults)."""
    nc = build_program()
    in_maps = make_in_maps(**inputs)
    res = run_bass_kernel_spmd(nc, in_maps, core_ids=list(range(NCORES)),
                               trace=trace)
    acc = np.zeros((S, HIDDEN), dtype=np.float32)
    for c in range(NCORES):
        acc += np.asarray(res.results[c]["out"], dtype=np.float32)
    return acc.reshape(1, S, HIDDEN), res


def kernel(**inputs) -> np.ndarray:
    out, _ = run(inputs)
    return out
